# revision 6
# baseline (speedup 1.0000x reference)
"""Tensor-parallel GQA attention forward for Trainium2, 8 NeuronCores.

Problem: nn_Attention (B=2, T=2048, D=4096, 32 q heads, 8 kv heads, hd=128).

Sharding (tensor-parallel over heads):
  - core c owns q heads 4c..4c+3 (512 features) and kv head c (128 features)
  - wq/wk/wv column-sharded, wo row-sharded; x replicated (pre-transposed on
    host to x^T [D, B*T] so projections need no on-device transpose)
  - each core returns its partial y @ wo_rows contribution; the host sums the
    8 partials (the unshard step for row-sharded wo).

All matmuls run in float32r (full-rate fp32 PE path, ~1.5e-4 rounding).

Device dataflow per core:
  P1: q^T/k^T/v^T = W^T x^T   (PSUM accum over 32 d-chunks), RoPE fused on
      q^T/k^T in [feat, token] layout via a host-permuted even/odd feature
      order (rotate-half becomes a 64-partition swap, done with cross-base
      DVE copies), v transposed to natural [token, d] tiles via PE transpose.
  P2: per (batch, head): scores^T = k^T.T @ q^T -> Exp (ScalarE, from PSUM)
      -> causal band mask (multiplicative) -> column sums via ones-matmul ->
      y~^T = v.T @ attn^T (PSUM accum) -> * (1/sums broadcast via K=1 matmul).
  P3: out_partial = y^T.T @ wo_rows  (PSUM accum over 4 feature chunks).
"""

import sys
import types

import numpy as np

B = 2
T = 2048
D = 4096
BT = B * T
NH = 32
NKV = 8
HD = 128
N_CORES = 8
QH = NH // N_CORES          # 4 q heads per core
QF = QH * HD                # 512 q features per core
KF = HD                     # 128 kv features per core
TCH = 256                   # phase-1 token chunk
NTC = BT // TCH             # 16 chunks
DC = D // 128               # 32 contraction chunks
QB = 512                    # phase-2 query block
SCALE = 1.0 / float(np.sqrt(HD))


def _install_ntff_hook_shim():
    """antenv.axon_hooks is absent in this image; synthesize it so
    run_bass_kernel_spmd(trace=True) can profile via libaxon_pjrt.so."""
    try:
        from antenv import axon_hooks  # noqa: F401
        return
    except ImportError:
        pass
    try:
        from trn_agent_boot.trn_boot import _ntff_profile_via_ctypes
        hook = _ntff_profile_via_ctypes("/opt/axon/libaxon_pjrt.so")
    except Exception:
        hook = None
    mod = types.ModuleType("antenv.axon_hooks")
    mod._hook = hook
    mod.get_axon_ntff_profile_hook = lambda: mod._hook

    def _set(h):
        mod._hook = h

    mod.set_axon_ntff_profile_hook = _set
    sys.modules["antenv.axon_hooks"] = mod


_install_ntff_hook_shim()

import concourse.bass as bass  # noqa: E402,F401
import concourse.bacc as bacc  # noqa: E402
import concourse.tile as tile  # noqa: E402
import concourse.mybir as mybir  # noqa: E402
from concourse import bass_utils  # noqa: E402
from concourse.masks import make_identity  # noqa: E402

F32 = mybir.dt.float32
F32R = mybir.dt.float32r
EXP = mybir.ActivationFunctionType.Exp

_NC_CACHE = []


def build():
    nc = bacc.Bacc("TRN2", target_bir_lowering=False, debug=False,
                   num_devices=N_CORES)

    xT = nc.dram_tensor("xT", [D, BT], F32R, kind="ExternalInput").ap()
    wq = nc.dram_tensor("wq", [D, QF], F32R, kind="ExternalInput").ap()
    wk = nc.dram_tensor("wk", [D, KF], F32R, kind="ExternalInput").ap()
    wv = nc.dram_tensor("wv", [D, KF], F32R, kind="ExternalInput").ap()
    wo = nc.dram_tensor("wo", [QF, D], F32R, kind="ExternalInput").ap()
    cc = nc.dram_tensor("cc", [128, BT], F32, kind="ExternalInput").ap()
    ss = nc.dram_tensor("ss", [128, BT], F32, kind="ExternalInput").ap()
    out = nc.dram_tensor("out", [BT, D], F32, kind="ExternalOutput").ap()

    with tile.TileContext(nc) as tc:
        _build_body(nc, tc, xT, wq, wk, wv, wo, cc, ss, out)
    nc.compile()
    return nc


def _build_body(nc, tc, xT, wq, wk, wv, wo, cc, ss, out):
    dram = tc.alloc_tile_pool(name="dram", bufs=1, space="DRAM")
    const = tc.alloc_tile_pool(name="const", bufs=1)
    # PSUM: 8 banks total -> 6 "big" [128,512] slots + 2 "small" [1,512]
    ps_big = tc.alloc_tile_pool(name="ps_big", bufs=6, space="PSUM")
    ps_small = tc.alloc_tile_pool(name="ps_small", bufs=2, space="PSUM")
    # phase-1 pools (released after phase 1 is emitted)
    wpool = tc.alloc_tile_pool(name="weights", bufs=1)
    xpool = tc.alloc_tile_pool(name="xstream", bufs=2)
    cspool = tc.alloc_tile_pool(name="cs", bufs=2)
    rpool = tc.alloc_tile_pool(name="rope", bufs=3)

    # ---- DRAM scratch (per-batch split for coarse phase overlap) ----
    qT_s = [dram.tile([QH, 128, T], F32R, tag=f"qTs{b}", name=f"qTs{b}") for b in range(B)]
    kT_s = [dram.tile([128, T], F32R, tag=f"kTs{b}", name=f"kTs{b}") for b in range(B)]
    v_s = [dram.tile([128, T // 128, 128], F32R, tag=f"vs{b}", name=f"vs{b}") for b in range(B)]
    yT_s = [dram.tile([QH, 128, T], F32R, tag=f"yTs{b}", name=f"yTs{b}") for b in range(B)]

    # ---- constants ----
    ident = const.tile([128, 128], F32)
    make_identity(nc, ident[:])
    onesP_f = const.tile([128, 1], F32)
    nc.vector.memset(onesP_f[:], 1.0)
    onesP = const.tile([128, 1], F32R)
    nc.vector.tensor_copy(onesP[:], onesP_f[:])
    ones1_f = const.tile([1, 128], F32)
    nc.vector.memset(ones1_f[:], 1.0)
    ones1 = const.tile([1, 128], F32R)
    nc.vector.tensor_copy(ones1[:], ones1_f[:])
    # BIG[p, v] = 1.0 iff v - 384 >= p ; mask(delta) = BIG[:, 384-delta :][:512]
    BIG = const.tile([128, 896], F32)
    nc.gpsimd.memset(BIG[:], 1.0)
    nc.gpsimd.affine_select(
        out=BIG[:], in_=BIG[:], compare_op=mybir.AluOpType.is_ge,
        fill=0.0, base=-384, channel_multiplier=-1, pattern=[[1, 896]],
    )

    # ---- phase 1: projections + RoPE ----
    wq_sb = wpool.tile([128, DC, QF], F32R)
    nc.sync.dma_start(wq_sb[:], wq.rearrange("(do di) f -> di do f", di=128))
    wk_sb = wpool.tile([128, DC, KF], F32R)
    nc.sync.dma_start(wk_sb[:], wk.rearrange("(do di) f -> di do f", di=128))
    wv_sb = wpool.tile([128, DC, KF], F32R)
    nc.sync.dma_start(wv_sb[:], wv.rearrange("(do di) f -> di do f", di=128))

    def rope_evict(ps, cc_t, ss_t, dst_ap):
        """psum [128, TCH] -> RoPE -> DMA to dst_ap (f32r)."""
        raw = rpool.tile([128, TCH], F32, tag="rraw")
        nc.any.tensor_copy(raw[:], ps[:])
        swp = rpool.tile([128, TCH], F32, tag="rswp")
        nc.vector.tensor_copy(swp[0:64, :], raw[64:128, :])
        nc.vector.tensor_copy(swp[64:128, :], raw[0:64, :])
        t1 = rpool.tile([128, TCH], F32R, tag="rt1")
        nc.vector.tensor_mul(out=swp[:], in0=swp[:], in1=ss_t[:])
        nc.vector.tensor_mul(out=t1[:], in0=raw[:], in1=cc_t[:])
        nc.vector.tensor_add(out=t1[:], in0=t1[:], in1=swp[:])
        nc.sync.dma_start(dst_ap, t1[:])

    for t in range(NTC):
        b, tloc = divmod(t * TCH, T)
        tsl = slice(t * TCH, (t + 1) * TCH)
        lsl = slice(tloc, tloc + TCH)
        xt = xpool.tile([128, DC, TCH], F32R)
        nc.sync.dma_start(
            xt[:], xT[:, tsl].rearrange("(do di) n -> di do n", di=128))
        cc_t = cspool.tile([128, TCH], F32, tag="cc")
        nc.sync.dma_start(cc_t[:], cc[:, tsl])
        ss_t = cspool.tile([128, TCH], F32, tag="ss")
        nc.sync.dma_start(ss_t[:], ss[:, tsl])

        for fc in range(QH):
            ps = ps_big.tile([128, 512], F32, tag="big", name="ps")[:, :TCH]
            for dc in range(DC):
                nc.tensor.matmul(
                    ps[:], wq_sb[:, dc, fc * 128:(fc + 1) * 128],
                    xt[:, dc, :], start=(dc == 0), stop=(dc == DC - 1))
            rope_evict(ps, cc_t, ss_t, qT_s[b][fc][:, lsl])

        ps = ps_big.tile([128, 512], F32, tag="big", name="ps")[:, :TCH]
        for dc in range(DC):
            nc.tensor.matmul(ps[:], wk_sb[:, dc, :], xt[:, dc, :],
                             start=(dc == 0), stop=(dc == DC - 1))
        rope_evict(ps, cc_t, ss_t, kT_s[b][:, lsl])

        ps = ps_big.tile([128, 512], F32, tag="big", name="ps")[:, :TCH]
        for dc in range(DC):
            nc.tensor.matmul(ps[:], wv_sb[:, dc, :], xt[:, dc, :],
                             start=(dc == 0), stop=(dc == DC - 1))
        vraw = rpool.tile([128, TCH], F32, tag="vraw")
        nc.any.tensor_copy(vraw[:], ps[:])
        for j in range(TCH // 128):
            pst = ps_big.tile([128, 512], F32, tag="big", name="pst")[:, :128]
            nc.tensor.transpose(pst[:], vraw[:, j * 128:(j + 1) * 128], ident[:])
            vt = rpool.tile([128, 128], F32R, tag="vt")
            nc.any.tensor_copy(vt[:], pst[:])
            g = (tloc // 128) + j
            nc.sync.dma_start(v_s[b][:, g, :], vt[:])

    rpool.release()
    cspool.release()
    xpool.release()
    wpool.release()
    kvpool = tc.alloc_tile_pool(name="kv", bufs=2)
    qpool = tc.alloc_tile_pool(name="q", bufs=2)
    apool = tc.alloc_tile_pool(name="attn", bufs=18)

    # ---- phase 2: attention per (batch, head) ----
    for b in range(B):
        kT_sb = kvpool.tile([128, T], F32R, tag="kT")
        nc.sync.dma_start(kT_sb[:], kT_s[b][:])
        v_sb = kvpool.tile([128, T // 128, 128], F32R, tag="v")
        nc.sync.dma_start(v_sb[:], v_s[b][:])
        for h in range(QH):
            for qb in range(T // QB):
                qT_sb = qpool.tile([128, QB], F32R, tag="qT")
                nc.sync.dma_start(
                    qT_sb[:], qT_s[b][h][:, qb * QB:(qb + 1) * QB])
                nkc = (qb + 1) * (QB // 128)
                atn = []
                for kc in range(nkc):
                    ps = ps_big.tile([128, QB], F32, tag="big")
                    nc.tensor.matmul(
                        ps[:], kT_sb[:, kc * 128:(kc + 1) * 128],
                        qT_sb[:], start=True, stop=True)
                    a = apool.tile([128, QB], F32R, tag="atn")
                    nc.scalar.activation(a[:], ps[:], EXP, scale=SCALE)
                    delta = kc * 128 - qb * QB
                    if delta >= 0:
                        off = 384 - delta
                        nc.vector.tensor_mul(
                            out=a[:], in0=a[:], in1=BIG[:, off:off + QB])
                    atn.append(a)
                ps_sm = ps_small.tile([1, QB], F32, tag="small")
                for kc in range(nkc):
                    nc.tensor.matmul(
                        ps_sm[:], onesP[:], atn[kc][:],
                        start=(kc == 0), stop=(kc == nkc - 1))
                rs_f = qpool.tile([1, QB], F32, tag="rsf")
                nc.vector.reciprocal(rs_f[:], ps_sm[:])
                rs = qpool.tile([1, QB], F32R, tag="rs")
                nc.vector.tensor_copy(rs[:], rs_f[:])
                ps_bc = ps_big.tile([128, QB], F32, tag="big")
                nc.tensor.matmul(ps_bc[:], ones1[:], rs[:],
                                 start=True, stop=True)
                rbc = qpool.tile([128, QB], F32, tag="rbc")
                nc.any.tensor_copy(rbc[:], ps_bc[:])
                ps_yt = ps_big.tile([128, QB], F32, tag="big")
                for kc in range(nkc):
                    nc.tensor.matmul(
                        ps_yt[:], v_sb[:, kc, :], atn[kc][:],
                        start=(kc == 0), stop=(kc == nkc - 1))
                yt = qpool.tile([128, QB], F32R, tag="yt")
                nc.vector.tensor_mul(out=yt[:], in0=ps_yt[:], in1=rbc[:])
                nc.sync.dma_start(yT_s[b][h][:, qb * QB:(qb + 1) * QB], yt[:])

    apool.release()
    qpool.release()
    kvpool.release()
    wopool = tc.alloc_tile_pool(name="wo", bufs=1)
    y4pool = tc.alloc_tile_pool(name="y4", bufs=2)
    opool = tc.alloc_tile_pool(name="outev", bufs=3)

    # ---- phase 3: out_partial = y @ wo_rows ----
    wo_sb = wopool.tile([128, QH, D], F32R)
    nc.sync.dma_start(wo_sb[:], wo.rearrange("(fo fi) n -> fi fo n", fi=128))
    for b in range(B):
        for tg in range(T // QB):
            y4 = y4pool.tile([128, QH, QB], F32R)
            for fc in range(QH):
                nc.sync.dma_start(
                    y4[:, fc, :], yT_s[b][fc][:, tg * QB:(tg + 1) * QB])
            for tcl in range(QB // 128):
                for oc in range(D // 512):
                    ps = ps_big.tile([128, 512], F32, tag="big")
                    for fc in range(QH):
                        nc.tensor.matmul(
                            ps[:],
                            y4[:, fc, tcl * 128:(tcl + 1) * 128],
                            wo_sb[:, fc, oc * 512:(oc + 1) * 512],
                            start=(fc == 0), stop=(fc == QH - 1))
                    ot = opool.tile([128, 512], F32, tag="ot")
                    nc.any.tensor_copy(ot[:], ps[:])
                    row0 = b * T + tg * QB + tcl * 128
                    nc.sync.dma_start(
                        out[row0:row0 + 128, oc * 512:(oc + 1) * 512], ot[:])

    opool.release()
    y4pool.release()
    wopool.release()
    ps_small.release()
    ps_big.release()
    const.release()
    dram.release()


_PERM = np.concatenate([np.arange(0, HD, 2), np.arange(1, HD, 2)])


def _prep_inputs(x, freqs_cis, wq, wk, wv, wo):
    x = np.asarray(x, dtype=np.float32)
    freqs_cis = np.asarray(freqs_cis, dtype=np.float32)
    wq = np.asarray(wq, dtype=np.float32)
    wk = np.asarray(wk, dtype=np.float32)
    wv = np.asarray(wv, dtype=np.float32)
    wo = np.asarray(wo, dtype=np.float32)

    xT = np.ascontiguousarray(x.reshape(BT, D).T)

    cosv = freqs_cis[:, :, 0].T                      # [64, T]
    sinv = freqs_cis[:, :, 1].T
    cc1 = np.concatenate([cosv, cosv], axis=0)       # [128, T]
    ss1 = np.concatenate([-sinv, sinv], axis=0)
    cc = np.ascontiguousarray(np.tile(cc1, (1, B)))  # [128, B*T]
    ss = np.ascontiguousarray(np.tile(ss1, (1, B)))

    in_maps = []
    for c in range(N_CORES):
        qcols = np.concatenate(
            [(4 * c + h) * HD + _PERM for h in range(QH)])
        kcols = c * HD + _PERM
        in_maps.append({
            "xT": xT,
            "wq": np.ascontiguousarray(wq[:, qcols]),
            "wk": np.ascontiguousarray(wk[:, kcols]),
            "wv": np.ascontiguousarray(wv[:, c * HD:(c + 1) * HD]),
            "wo": np.ascontiguousarray(wo[c * QF:(c + 1) * QF, :]),
            "cc": cc,
            "ss": ss,
        })
    return in_maps


def kernel(x, freqs_cis, wq, wk, wv, wo):
    if not _NC_CACHE:
        _NC_CACHE.append(build())
    nc = _NC_CACHE[0]
    in_maps = _prep_inputs(x, freqs_cis, wq, wk, wv, wo)
    res = bass_utils.run_bass_kernel_spmd(
        nc, in_maps, core_ids=list(range(N_CORES)))
    acc = res.results[0]["out"].astype(np.float32, copy=True)
    for i in range(1, N_CORES):
        acc += res.results[i]["out"]
    return acc.reshape(B, T, D)


if __name__ == "__main__":
    rng = np.random.default_rng(0)
    s = 1.0 / np.sqrt(D)
    inputs = {
        "x": rng.standard_normal((B, T, D), dtype=np.float32),
        "freqs_cis": rng.standard_normal((T, HD // 2, 2), dtype=np.float32),
        "wq": rng.standard_normal((D, NH * HD), dtype=np.float32) * s,
        "wk": rng.standard_normal((D, NKV * HD), dtype=np.float32) * s,
        "wv": rng.standard_normal((D, NKV * HD), dtype=np.float32) * s,
        "wo": rng.standard_normal((D, D), dtype=np.float32) * s,
    }
    out = kernel(**inputs)
    print("out", out.shape, out.dtype, float(np.abs(out).mean()))


# revision 7
# speedup vs baseline: 1.4212x; 1.4212x over previous
"""Tensor-parallel GQA attention forward for Trainium2, 8 NeuronCores.

Problem: nn_Attention (B=2, T=2048, D=4096, 32 q heads, 8 kv heads, hd=128).

Sharding (tensor-parallel over heads):
  - core c owns q heads 4c..4c+3 (512 features) and kv head c (128 features)
  - wq/wk/wv column-sharded, wo row-sharded; x replicated (pre-transposed on
    host to x^T [D, B*T] so projections need no on-device transpose)
  - each core returns its partial y @ wo_rows contribution; the host sums the
    8 partials (the unshard step for row-sharded wo).

All matmuls run in float32r (full-rate fp32 PE path, ~1.5e-4 rounding).

Device dataflow per core:
  P1: q^T/k^T/v^T = W^T x^T   (PSUM accum over 32 d-chunks), RoPE fused on
      q^T/k^T in [feat, token] layout via a host-permuted even/odd feature
      order (rotate-half becomes a 64-partition swap, done with cross-base
      DVE copies), v transposed to natural [token, d] tiles via PE transpose.
  P2: per (batch, head): scores^T = k^T.T @ q^T -> Exp (ScalarE, from PSUM)
      -> causal band mask (multiplicative) -> column sums via ones-matmul ->
      y~^T = v.T @ attn^T (PSUM accum) -> * (1/sums broadcast via K=1 matmul).
  P3: out_partial = y^T.T @ wo_rows  (PSUM accum over 4 feature chunks).
"""

import sys
import types

import numpy as np

B = 2
T = 2048
D = 4096
BT = B * T
NH = 32
NKV = 8
HD = 128
N_CORES = 8
QH = NH // N_CORES          # 4 q heads per core
QF = QH * HD                # 512 q features per core
KF = HD                     # 128 kv features per core
TCH = 256                   # phase-1 token chunk
NTC = BT // TCH             # 16 chunks
DC = D // 128               # 32 contraction chunks
QB = 512                    # phase-2 query block
SCALE = 1.0 / float(np.sqrt(HD))


def _install_ntff_hook_shim():
    """antenv.axon_hooks is absent in this image; synthesize it so
    run_bass_kernel_spmd(trace=True) can profile via libaxon_pjrt.so."""
    try:
        from antenv import axon_hooks  # noqa: F401
        return
    except ImportError:
        pass
    try:
        from trn_agent_boot.trn_boot import _ntff_profile_via_ctypes
        hook = _ntff_profile_via_ctypes("/opt/axon/libaxon_pjrt.so")
    except Exception:
        hook = None
    mod = types.ModuleType("antenv.axon_hooks")
    mod._hook = hook
    mod.get_axon_ntff_profile_hook = lambda: mod._hook

    def _set(h):
        mod._hook = h

    mod.set_axon_ntff_profile_hook = _set
    sys.modules["antenv.axon_hooks"] = mod


_install_ntff_hook_shim()

import concourse.bass as bass  # noqa: E402,F401
import concourse.bacc as bacc  # noqa: E402
import concourse.tile as tile  # noqa: E402
import concourse.mybir as mybir  # noqa: E402
from concourse import bass_utils  # noqa: E402
from concourse.masks import make_identity  # noqa: E402

F32 = mybir.dt.float32
F32R = mybir.dt.float32r
EXP = mybir.ActivationFunctionType.Exp

_NC_CACHE = []


def build():
    nc = bacc.Bacc("TRN2", target_bir_lowering=False, debug=False,
                   num_devices=N_CORES)

    xT = nc.dram_tensor("xT", [128, NTC, DC, TCH], F32R, kind="ExternalInput").ap()
    wq = nc.dram_tensor("wq", [128, DC, QF], F32R, kind="ExternalInput").ap()
    wk = nc.dram_tensor("wk", [128, DC, KF], F32R, kind="ExternalInput").ap()
    wv = nc.dram_tensor("wv", [128, DC, KF], F32R, kind="ExternalInput").ap()
    wo = nc.dram_tensor("wo", [128, QH, D], F32R, kind="ExternalInput").ap()
    cc = nc.dram_tensor("cc", [128, BT], F32, kind="ExternalInput").ap()
    ss = nc.dram_tensor("ss", [128, BT], F32, kind="ExternalInput").ap()
    out = nc.dram_tensor("out", [BT, D], F32, kind="ExternalOutput").ap()

    with tile.TileContext(nc) as tc:
        _build_body(nc, tc, xT, wq, wk, wv, wo, cc, ss, out)
    nc.compile()
    return nc


def _build_body(nc, tc, xT, wq, wk, wv, wo, cc, ss, out):
    dram = tc.alloc_tile_pool(name="dram", bufs=1, space="DRAM")
    const = tc.alloc_tile_pool(name="const", bufs=1)
    # PSUM: 8 banks total -> 6 "big" [128,512] slots + 2 "small" [1,512]
    ps_big = tc.alloc_tile_pool(name="ps_big", bufs=8, space="PSUM")
    # phase-1 pools (released after phase 1 is emitted)
    wpool = tc.alloc_tile_pool(name="weights", bufs=1)
    xpool = tc.alloc_tile_pool(name="xstream", bufs=2)
    cspool = tc.alloc_tile_pool(name="cs", bufs=2)
    rpool = tc.alloc_tile_pool(name="rope", bufs=3)

    # ---- DRAM scratch (per-batch split for coarse phase overlap) ----
    qT_s = [dram.tile([QH, 128, T], F32R, tag=f"qTs{b}", name=f"qTs{b}") for b in range(B)]
    kT_s = [dram.tile([128, T], F32R, tag=f"kTs{b}", name=f"kTs{b}") for b in range(B)]
    v_s = [dram.tile([128, T // 128, 128], F32R, tag=f"vs{b}", name=f"vs{b}") for b in range(B)]
    yT_s = [dram.tile([QH, 128, T], F32R, tag=f"yTs{b}", name=f"yTs{b}") for b in range(B)]

    # ---- constants ----
    ident = const.tile([128, 128], F32)
    make_identity(nc, ident[:])
    onesPP_f = const.tile([128, 128], F32)
    nc.vector.memset(onesPP_f[:], 1.0)
    onesPP = const.tile([128, 128], F32R)
    nc.vector.tensor_copy(onesPP[:], onesPP_f[:])
    # BIG[p, v] = 1.0 iff v - 384 >= p ; mask(delta) = BIG[:, 384-delta :][:512]
    BIG = const.tile([128, 896], F32)
    nc.gpsimd.memset(BIG[:], 1.0)
    nc.gpsimd.affine_select(
        out=BIG[:], in_=BIG[:], compare_op=mybir.AluOpType.is_ge,
        fill=0.0, base=-384, channel_multiplier=-1, pattern=[[1, 896]],
    )

    # ---- phase 1: projections + RoPE ----
    wq_sb = wpool.tile([128, DC, QF], F32R)
    nc.sync.dma_start(wq_sb[:], wq[:])
    wk_sb = wpool.tile([128, DC, KF], F32R)
    nc.sync.dma_start(wk_sb[:], wk[:])
    wv_sb = wpool.tile([128, DC, KF], F32R)
    nc.sync.dma_start(wv_sb[:], wv[:])

    def rope_evict(ps, cc_t, ss_t, dst_ap):
        """psum [128, TCH] -> RoPE -> DMA to dst_ap (f32r)."""
        raw = rpool.tile([128, TCH], F32, tag="rraw")
        nc.any.tensor_copy(raw[:], ps[:])
        swp = rpool.tile([128, TCH], F32, tag="rswp")
        nc.vector.tensor_copy(swp[0:64, :], raw[64:128, :])
        nc.vector.tensor_copy(swp[64:128, :], raw[0:64, :])
        t1 = rpool.tile([128, TCH], F32R, tag="rt1")
        nc.vector.tensor_mul(out=swp[:], in0=swp[:], in1=ss_t[:])
        nc.vector.tensor_mul(out=t1[:], in0=raw[:], in1=cc_t[:])
        nc.vector.tensor_add(out=t1[:], in0=t1[:], in1=swp[:])
        nc.sync.dma_start(dst_ap, t1[:])

    for t in range(NTC):
        b, tloc = divmod(t * TCH, T)
        tsl = slice(t * TCH, (t + 1) * TCH)
        lsl = slice(tloc, tloc + TCH)
        xt = xpool.tile([128, DC, TCH], F32R)
        nc.sync.dma_start(xt[:], xT[:, t, :, :])
        cc_t = cspool.tile([128, TCH], F32, tag="cc")
        nc.sync.dma_start(cc_t[:], cc[:, tsl])
        ss_t = cspool.tile([128, TCH], F32, tag="ss")
        nc.sync.dma_start(ss_t[:], ss[:, tsl])

        for fc in range(QH):
            ps = ps_big.tile([128, 512], F32, tag="big", name="ps")[:, :TCH]
            for dc in range(DC):
                nc.tensor.matmul(
                    ps[:], wq_sb[:, dc, fc * 128:(fc + 1) * 128],
                    xt[:, dc, :], start=(dc == 0), stop=(dc == DC - 1))
            rope_evict(ps, cc_t, ss_t, qT_s[b][fc][:, lsl])

        ps = ps_big.tile([128, 512], F32, tag="big", name="ps")[:, :TCH]
        for dc in range(DC):
            nc.tensor.matmul(ps[:], wk_sb[:, dc, :], xt[:, dc, :],
                             start=(dc == 0), stop=(dc == DC - 1))
        rope_evict(ps, cc_t, ss_t, kT_s[b][:, lsl])

        ps = ps_big.tile([128, 512], F32, tag="big", name="ps")[:, :TCH]
        for dc in range(DC):
            nc.tensor.matmul(ps[:], wv_sb[:, dc, :], xt[:, dc, :],
                             start=(dc == 0), stop=(dc == DC - 1))
        vraw = rpool.tile([128, TCH], F32, tag="vraw")
        nc.any.tensor_copy(vraw[:], ps[:])
        for j in range(TCH // 128):
            pst = ps_big.tile([128, 512], F32, tag="big", name="pst")[:, :128]
            nc.tensor.transpose(pst[:], vraw[:, j * 128:(j + 1) * 128], ident[:])
            vt = rpool.tile([128, 128], F32R, tag="vt")
            nc.any.tensor_copy(vt[:], pst[:])
            g = (tloc // 128) + j
            nc.sync.dma_start(v_s[b][:, g, :], vt[:])

    rpool.release()
    cspool.release()
    xpool.release()
    wpool.release()
    wopool = tc.alloc_tile_pool(name="wo", bufs=1)
    wo_sb = wopool.tile([128, QH, D], F32R)
    nc.sync.dma_start(wo_sb[:], wo[:])
    kvpool = tc.alloc_tile_pool(name="kv", bufs=2)
    qpool = tc.alloc_tile_pool(name="q", bufs=2)
    apool = tc.alloc_tile_pool(name="attn", bufs=18)

    # ---- phase 2: attention per (batch, head) ----
    for b in range(B):
        kT_sb = kvpool.tile([128, T], F32R, tag="kT")
        nc.sync.dma_start(kT_sb[:], kT_s[b][:])
        v_sb = kvpool.tile([128, T // 128, 128], F32R, tag="v")
        nc.sync.dma_start(v_sb[:], v_s[b][:])
        for h in range(QH):
            for qb in range(T // QB):
                qT_sb = qpool.tile([128, QB], F32R, tag="qT")
                nc.sync.dma_start(
                    qT_sb[:], qT_s[b][h][:, qb * QB:(qb + 1) * QB])
                nkc = (qb + 1) * (QB // 128)
                atn = []
                for kc in range(nkc):
                    ps = ps_big.tile([128, QB], F32, tag="big")
                    nc.tensor.matmul(
                        ps[:], kT_sb[:, kc * 128:(kc + 1) * 128],
                        qT_sb[:], start=True, stop=True)
                    a = apool.tile([128, QB], F32R, tag="atn")
                    nc.scalar.activation(a[:], ps[:], EXP, scale=SCALE)
                    delta = kc * 128 - qb * QB
                    if delta >= 0:
                        off = 384 - delta
                        nc.vector.tensor_mul(
                            out=a[:], in0=a[:], in1=BIG[:, off:off + QB])
                    atn.append(a)
                ps_yt = ps_big.tile([128, QB], F32, tag="big")
                for kc in range(nkc):
                    nc.tensor.matmul(
                        ps_yt[:], v_sb[:, kc, :], atn[kc][:],
                        start=(kc == 0), stop=(kc == nkc - 1))
                ps_bc = ps_big.tile([128, QB], F32, tag="big")
                for kc in range(nkc):
                    nc.tensor.matmul(
                        ps_bc[:], onesPP[:], atn[kc][:],
                        start=(kc == 0), stop=(kc == nkc - 1))
                ssb = qpool.tile([128, QB], F32, tag="ssb")
                nc.any.tensor_copy(ssb[:], ps_bc[:])
                rb = qpool.tile([128, QB], F32, tag="rb")
                nc.vector.reciprocal_approx_fast(out=rb[:], in_=ssb[:])
                yt = qpool.tile([128, QB], F32R, tag="yt")
                nc.vector.tensor_mul(out=yt[:], in0=ps_yt[:], in1=rb[:])
                nc.sync.dma_start(yT_s[b][h][:, qb * QB:(qb + 1) * QB], yt[:])

    apool.release()
    qpool.release()
    kvpool.release()
    y4pool = tc.alloc_tile_pool(name="y4", bufs=2)
    opool = tc.alloc_tile_pool(name="outev", bufs=3)

    # ---- phase 3: out_partial = y @ wo_rows ----
    for b in range(B):
        for tg in range(T // QB):
            y4 = y4pool.tile([128, QH, QB], F32R)
            for fc in range(QH):
                nc.sync.dma_start(
                    y4[:, fc, :], yT_s[b][fc][:, tg * QB:(tg + 1) * QB])
            for tcl in range(QB // 128):
                for oc in range(D // 512):
                    ps = ps_big.tile([128, 512], F32, tag="big")
                    for fc in range(QH):
                        nc.tensor.matmul(
                            ps[:],
                            y4[:, fc, tcl * 128:(tcl + 1) * 128],
                            wo_sb[:, fc, oc * 512:(oc + 1) * 512],
                            start=(fc == 0), stop=(fc == QH - 1))
                    ot = opool.tile([128, 512], F32, tag="ot")
                    nc.vector.tensor_copy(ot[:], ps[:])
                    row0 = b * T + tg * QB + tcl * 128
                    nc.sync.dma_start(
                        out[row0:row0 + 128, oc * 512:(oc + 1) * 512], ot[:])

    opool.release()
    y4pool.release()
    wopool.release()
    ps_big.release()
    const.release()
    dram.release()


_PERM = np.concatenate([np.arange(0, HD, 2), np.arange(1, HD, 2)])


def _prep_inputs(x, freqs_cis, wq, wk, wv, wo):
    x = np.asarray(x, dtype=np.float32)
    freqs_cis = np.asarray(freqs_cis, dtype=np.float32)
    wq = np.asarray(wq, dtype=np.float32)
    wk = np.asarray(wk, dtype=np.float32)
    wv = np.asarray(wv, dtype=np.float32)
    wo = np.asarray(wo, dtype=np.float32)

    x2 = x.reshape(BT, D)
    # [di, tchunk, dc, tlocal] so each phase-1 chunk DMA is 128 x 32KB contig
    xTq = np.ascontiguousarray(
        x2.reshape(NTC, TCH, DC, 128).transpose(3, 0, 2, 1))

    cosv = freqs_cis[:, :, 0].T                      # [64, T]
    sinv = freqs_cis[:, :, 1].T
    cc1 = np.concatenate([cosv, cosv], axis=0)       # [128, T]
    ss1 = np.concatenate([-sinv, sinv], axis=0)
    cc = np.ascontiguousarray(np.tile(cc1, (1, B)))  # [128, B*T]
    ss = np.ascontiguousarray(np.tile(ss1, (1, B)))

    in_maps = []
    for c in range(N_CORES):
        qcols = np.concatenate(
            [(4 * c + h) * HD + _PERM for h in range(QH)])
        kcols = c * HD + _PERM
        in_maps.append({
            "xT": xTq,
            "wq": np.ascontiguousarray(
                wq[:, qcols].reshape(DC, 128, QF).transpose(1, 0, 2)),
            "wk": np.ascontiguousarray(
                wk[:, kcols].reshape(DC, 128, KF).transpose(1, 0, 2)),
            "wv": np.ascontiguousarray(
                wv[:, c * HD:(c + 1) * HD].reshape(DC, 128, KF)
                .transpose(1, 0, 2)),
            "wo": np.ascontiguousarray(
                wo[c * QF:(c + 1) * QF, :].reshape(QH, 128, D)
                .transpose(1, 0, 2)),
            "cc": cc,
            "ss": ss,
        })
    return in_maps


def kernel(x, freqs_cis, wq, wk, wv, wo):
    if not _NC_CACHE:
        _NC_CACHE.append(build())
    nc = _NC_CACHE[0]
    in_maps = _prep_inputs(x, freqs_cis, wq, wk, wv, wo)
    res = bass_utils.run_bass_kernel_spmd(
        nc, in_maps, core_ids=list(range(N_CORES)))
    acc = res.results[0]["out"].astype(np.float32, copy=True)
    for i in range(1, N_CORES):
        acc += res.results[i]["out"]
    return acc.reshape(B, T, D)


if __name__ == "__main__":
    rng = np.random.default_rng(0)
    s = 1.0 / np.sqrt(D)
    inputs = {
        "x": rng.standard_normal((B, T, D), dtype=np.float32),
        "freqs_cis": rng.standard_normal((T, HD // 2, 2), dtype=np.float32),
        "wq": rng.standard_normal((D, NH * HD), dtype=np.float32) * s,
        "wk": rng.standard_normal((D, NKV * HD), dtype=np.float32) * s,
        "wv": rng.standard_normal((D, NKV * HD), dtype=np.float32) * s,
        "wo": rng.standard_normal((D, D), dtype=np.float32) * s,
    }
    out = kernel(**inputs)
    print("out", out.shape, out.dtype, float(np.abs(out).mean()))
